# revision 18
# baseline (speedup 1.0000x reference)
"""Trainium2 Bass kernel for nn_Attention_11991548690893.

Reference semantics (faithfully-reproduced bug): q = k = v = the key
projection, so only the middle third of W_attn is used and the attention
matrix S = kh @ kh^T is SYMMETRIC.  We exploit:
  - Megatron head-sharding: core c owns heads 2c, 2c+1 (128 head-dims),
    computes a partial output against its 128 rows of W_proj; the host
    sums the 8 partials and adds b_proj.
  - Host-side transpose of x -> xT so the on-device k projection needs
    no transposes of the big activation.
  - Symmetry of S: exp(S) row-blocks serve directly as [k-part, q-free]
    operands for the second attention matmul (no transpose).
  - Softmax without max-subtraction (logits are bounded ~15 here; fp32
    exp is safe) with rowsum from the ACT accum_out port.
  - fp32r matmuls (single-pass fp32 on the PE, 1 cycle/row at N>=512).
"""

import numpy as np

import concourse.bass as bass
import concourse.mybir as mybir
import concourse.tile as tile
from concourse import bacc
from concourse.bass_utils import run_bass_kernel_spmd

F32 = mybir.dt.float32
F32R = mybir.dt.float32r

B = 2
L = 2048
D = 1024
H = 16
DH = 64
NCORES = 8
DHC = 128            # head-dims per core (2 heads x 64)
L2 = B * L           # 4096
P = 128
NBLK = L // P        # 16 l-blocks per batch
SCALE = 1.0 / np.sqrt(DH)   # 0.125


def _build_kernel(ctx, tc, xT, wk, bk, wp, ident_dram, out):
    nc = tc.nc

    singles = ctx.enter_context(tc.tile_pool(name="singles", bufs=1))
    xpool = ctx.enter_context(tc.tile_pool(name="xpool", bufs=3))
    spool = ctx.enter_context(tc.tile_pool(name="spool", bufs=3))
    rpool = ctx.enter_context(tc.tile_pool(name="rpool", bufs=2))
    otpool = ctx.enter_context(tc.tile_pool(name="otpool", bufs=1))
    opool = ctx.enter_context(tc.tile_pool(name="opool", bufs=3))
    ps_main = ctx.enter_context(tc.tile_pool(name="ps_main", bufs=2, space="PSUM"))
    ps_out = ctx.enter_context(tc.tile_pool(name="ps_out", bufs=1, space="PSUM"))
    dpool = ctx.enter_context(tc.tile_pool(name="dpool", bufs=2, space="DRAM"))

    ident = singles.tile([P, P], F32R)
    nc.sync.dma_start(ident, ident_dram)

    wk_sb = singles.tile([P, 8, DHC], F32R)   # W_k slice, D-major tiles
    nc.sync.dma_start(wk_sb, wk.rearrange("(o p) m -> p o m", p=P))
    bk_sb = singles.tile([P, 1], F32)
    nc.sync.dma_start(bk_sb, bk)
    wp_sb = singles.tile([DH, 2, D], F32R)   # W_proj rows split per head
    nc.sync.dma_start(wp_sb, wp.rearrange("(t p) d -> p t d", p=DH))

    # ---- Phase 1: kT chunks = (x @ Wk + bk)^T, [128 dh, 512 l] x 8 ----
    xTr = xT.rearrange("(o p) l -> p o l", p=P)   # [128, 8, 4096]
    kT = []                                       # 8 x [128, 512]
    for lc in range(8):
        xc = xpool.tile([P, 8, 512], F32R, tag="xc")
        nc.sync.dma_start(xc, xTr[:, :, lc * 512:(lc + 1) * 512])
        ps = ps_main.tile([P, 512], F32, tag="mm")
        for dc in range(8):
            nc.tensor.matmul(
                ps,
                wk_sb[:, dc],
                xc[:, dc],
                start=(dc == 0),
                stop=(dc == 7),
            )
        kt = singles.tile([P, 512], F32R, tag=f"kt{lc}")
        nc.vector.tensor_scalar_add(kt, ps, bk_sb)
        kT.append(kt)

    # ---- k natural blocks + ones cols: knat [128 l, 32 blk, 130] ----
    # per block: [0:64]=head A, 64=ones, [65:129]=head B, 129=ones, so
    # [:, i, 65*h2 : 65*h2+65] is [kh_block | 1] — the out^T stationary
    # whose last column accumulates the softmax denominators.
    knat = singles.tile([P, 32, 2, P], F32R)
    ones32 = singles.tile([P, 32], F32)
    nc.vector.memset(ones32, 1.0)
    nc.vector.tensor_copy(knat[:, :, 0, 64:65], ones32.unsqueeze(-1))
    nc.vector.tensor_copy(knat[:, :, 1, 64:65], ones32.unsqueeze(-1))
    zpad = singles.tile([P, 2, 63], F32)
    nc.vector.memset(zpad, 0.0)
    for i in range(32):
        nc.vector.tensor_copy(knat[:, i, :, 65:], zpad)
    for i in range(32):
        tps = ps_main.tile([P, P], F32R, tag="mm")
        nc.tensor.transpose(tps, kT[i // 4][:, (i % 4) * P:(i % 4 + 1) * P], ident)
        nc.vector.tensor_copy(knat[:, i, 0, 0:64], tps[:, 0:64])
        nc.vector.tensor_copy(knat[:, i, 1, 0:64], tps[:, 64:128])

    def khT_chunk(b_, h2, c512):
        """[64, 512] slice of kT for batch b_, in-core head h2, l-chunk c512."""
        t = kT[b_ * 4 + c512]
        return t[h2 * DH:(h2 + 1) * DH, :]

    # ---- Phase 2: attention per batch, 2 heads; out^T accumulated in PSUM ----
    for b_ in range(B):
        oT_sb = []
        for h2 in range(2):
            oT_ps = ps_out.tile([P, L], F32, tag="ot")   # rows 0:64 out^T, row 64 denom, 65+ pad
            for i in range(NBLK):
                # stationary [64, 128]: q-block i of khT
                lhsT_att = kT[b_ * 4 + i // 4][
                    h2 * DH:(h2 + 1) * DH, (i % 4) * P:(i % 4 + 1) * P
                ]
                Sb = spool.tile([P, L], F32R, tag="S")    # raw exp(S) row-block
                for kc in range(2):
                    aps = ps_main.tile([P, 1024], F32, tag="mm")
                    for n2 in range(2):
                        nc.tensor.matmul(
                            aps[:, n2 * 512:(n2 + 1) * 512],
                            lhsT_att,
                            khT_chunk(b_, h2, kc * 2 + n2),
                            start=True,
                            stop=True,
                        )
                    nc.scalar.activation(
                        Sb[:, kc * 1024:(kc + 1) * 1024],
                        aps,
                        mybir.ActivationFunctionType.Exp,
                        scale=SCALE,
                    )
                # [out^T | denom] += [kh_blk | 1]^T @ expS_blk (S symmetric)
                lhsT_o = knat[:, b_ * NBLK + i, h2]
                for qc in range(4):
                    nc.tensor.matmul(
                        oT_ps[:, qc * 512:(qc + 1) * 512],
                        lhsT_o,
                        Sb[:, qc * 512:(qc + 1) * 512],
                        start=(i == 0),
                        stop=(i == NBLK - 1),
                        skip_group_check=True,
                    )
            # normalize: out^T row-block / denom (broadcast along partitions)
            recip = rpool.tile([1, L], F32, tag="recip")
            nc.vector.reciprocal(recip, oT_ps[DH:DH + 1, :])
            rdram = dpool.tile([1, L], F32)
            nc.sync.dma_start(rdram, recip)
            bcast = otpool.tile([DH, L], F32, tag="bc")
            nc.sync.dma_start(
                bcast,
                bass.AP(tensor=rdram.tensor, offset=rdram.offset,
                        ap=[[0, DH]] + list(rdram.ap)[1:]),
            )
            osb_h = otpool.tile([DH, L], F32R, tag=f"oT{h2}")
            nc.vector.tensor_mul(osb_h, oT_ps[0:DH, :], bcast)
            oT_sb.append(osb_h)
        # ---- Phase 3: partial = sum_h out_h^T.T @ Wp_h (two K=64 matmuls) ----
        for qt in range(NBLK):
            pps = ps_main.tile([P, 1024], F32, tag="mm")
            for n2 in range(2):
                for h2 in range(2):
                    nc.tensor.matmul(
                        pps[:, n2 * 512:(n2 + 1) * 512],
                        oT_sb[h2][:, qt * P:(qt + 1) * P],
                        wp_sb[:, h2, n2 * 512:(n2 + 1) * 512],
                        start=(h2 == 0),
                        stop=(h2 == 1),
                    )
            osb = opool.tile([P, D], F32, tag="osb")
            nc.vector.tensor_copy(osb, pps)
            nc.sync.dma_start(out[b_ * L + qt * P: b_ * L + (qt + 1) * P, :], osb)


_NC_CACHE = None


def _get_nc():
    global _NC_CACHE
    if _NC_CACHE is None:
        nc = bacc.Bacc("TRN2", target_bir_lowering=False)
        xT = nc.dram_tensor("xt", [D, L2], F32R, kind="ExternalInput").ap()
        wk = nc.dram_tensor("wk", [D, DHC], F32R, kind="ExternalInput").ap()
        bk = nc.dram_tensor("bk", [DHC, 1], F32, kind="ExternalInput").ap()
        wp = nc.dram_tensor("wp", [DHC, D], F32R, kind="ExternalInput").ap()
        ident = nc.dram_tensor("ident", [P, P], F32R, kind="ExternalInput").ap()
        out = nc.dram_tensor("out", [L2, D], F32, kind="ExternalOutput").ap()
        from contextlib import ExitStack
        with tile.TileContext(nc) as tc, ExitStack() as ctx:
            _build_kernel(ctx, tc, xT, wk, bk, wp, ident, out)
        nc.compile()
        _NC_CACHE = nc
    return _NC_CACHE


def _run(inputs, trace=False):
    x = np.asarray(inputs["x"], dtype=np.float32)
    W_attn = np.asarray(inputs["W_attn"], dtype=np.float32)
    b_attn = np.asarray(inputs["b_attn"], dtype=np.float32)
    W_proj = np.asarray(inputs["W_proj"], dtype=np.float32)
    b_proj = np.asarray(inputs["b_proj"], dtype=np.float32)

    xT = np.ascontiguousarray(x.reshape(L2, D).T)           # [1024, 4096]
    Wk = W_attn[:, D:2 * D]                                  # [1024, 1024]
    bk = b_attn[D:2 * D]                                     # [1024]

    in_maps = []
    for c in range(NCORES):
        sl = slice(c * DHC, (c + 1) * DHC)
        in_maps.append({
            "xt": xT,
            "wk": np.ascontiguousarray(Wk[:, sl]),
            "bk": np.ascontiguousarray(bk[sl]).reshape(DHC, 1),
            "wp": np.ascontiguousarray(W_proj[sl, :]),
            "ident": np.eye(P, dtype=np.float32),
        })

    nc = _get_nc()
    res = run_bass_kernel_spmd(nc, in_maps, core_ids=list(range(NCORES)),
                               trace=trace)
    acc = res.results[0]["out"].astype(np.float64)
    for r in res.results[1:]:
        acc += r["out"]
    acc += b_proj
    return acc.astype(np.float32).reshape(B, L, D), res


def kernel(**inputs):
    out, _ = _run(inputs, trace=False)
    return out


def kernel_traced(**inputs):
    return _run(inputs, trace=True)
